# revision 5
# baseline (speedup 1.0000x reference)
"""GQA attention block (B=2,S=2048,EMB=2048,H=32,G=8,D=64) on 8 trn2 cores.

Sharding: DP over batch (2) x TP over heads (4). Each core handles one batch
and 8 Q heads / 2 KV groups. Wq/Wk/Wv column-sharded, Wo row-sharded with an
on-device ReduceScatter over each batch's 4-core group; host concatenates the
disjoint row slices.

All heavy matmuls run in float32r (TF32-like, ~11-bit mantissa, full PE rate).
Softmax needs no running max: rmsnorm bounds |logits| <= 8.
"""

import numpy as np

import concourse.bass as bass
import concourse.mybir as mybir
import concourse.tile as tile
from concourse import bacc
from concourse.bass_utils import run_bass_kernel_spmd

F32 = mybir.dt.float32
F32R = mybir.dt.float32r
AF = mybir.ActivationFunctionType

B, S, EMB, H, G, D = 2, 2048, 2048, 32, 8, 64
TP = 4                      # tensor-parallel degree (cores per batch)
HL = H // TP                # 8 local heads
NPAIR = HL // 2             # 4 head pairs (one per 128-feat block)
S1 = 256                    # phase-1 token strip
NS1 = S // S1               # 8
S2 = 512                    # attention query strip
NS2 = S // S2               # 4
KB = S // 128               # 16 key blocks
NEG = -30.0

_CACHE = {}
LAST_RESULT = None


def _build():
    nc = bacc.Bacc(None, target_bir_lowering=False, debug=False)

    xT_in = nc.dram_tensor("xT", [EMB, S], F32R, kind="ExternalInput")
    wq_in = nc.dram_tensor("wq", [EMB, 512], F32R, kind="ExternalInput")
    wkv_in = nc.dram_tensor("wkv", [EMB, 256], F32R, kind="ExternalInput")
    wo_in = nc.dram_tensor("wo", [512, EMB], F32R, kind="ExternalInput")
    cos_in = nc.dram_tensor("cosT", [64, S], F32, kind="ExternalInput")
    sin_in = nc.dram_tensor("sinP", [64, S], F32, kind="ExternalInput")
    pm_in = nc.dram_tensor("pm", [64, 64], F32R, kind="ExternalInput")
    qsr_in = nc.dram_tensor("qsr", [1, 64], F32R, kind="ExternalInput")
    ksr_in = nc.dram_tensor("ksr", [1, 64], F32R, kind="ExternalInput")
    ind_in = nc.dram_tensor("ind", [NPAIR, 2, 64, 8], F32R, kind="ExternalInput")
    indk_in = nc.dram_tensor("indk", [2, 64, 2], F32R, kind="ExternalInput")
    ones_in = nc.dram_tensor("ones64", [1, 64], F32R, kind="ExternalInput")
    vones_in = nc.dram_tensor("vones", [128, KB], F32R, kind="ExternalInput")
    mask_in = nc.dram_tensor("maskadd", [4, 128, S2], F32, kind="ExternalInput")
    id_in = nc.dram_tensor("ident", [128, 128], F32R, kind="ExternalInput")
    out_ext = nc.dram_tensor("myout", [NS2, 128, EMB], F32, kind="ExternalOutput")

    with tile.TileContext(nc) as tc:
        # ---------- long-lived tensors ----------
        with tc.tile_pool(name="resident", bufs=1) as pres, \
             tc.tile_pool(name="dram", bufs=1, space="DRAM") as pdram:
            qTn = [pres.tile([128, S], F32R, name=f"qTn{m}") for m in range(NPAIR)]
            kTn = pres.tile([128, S], F32R)
            Vg = [pres.tile([128, KB, 65], F32R, name=f"Vg{g}") for g in range(2)]
            ones64 = pres.tile([1, 64], F32R)
            nc.sync.dma_start(out=ones64[:], in_=ones_in[:])
            for g in range(2):
                nc.sync.dma_start(out=Vg[g][:, :, 64:65], in_=vones_in[:, :, None])
            oPart = pdram.tile([S, EMB], F32)
            rsOut = pdram.tile([NS2, 128, EMB], F32)

            # ================= Phase 1: projections + norm + rope ========
            with tc.tile_pool(name="p1w", bufs=1) as p1w, \
                 tc.tile_pool(name="p1x", bufs=2) as p1x, \
                 tc.tile_pool(name="p1c", bufs=1) as p1c, \
                 tc.tile_pool(name="p1t", bufs=2) as p1t, \
                 tc.tile_pool(name="ps_proj", bufs=2, space="PSUM") as ps_proj, \
                 tc.tile_pool(name="ps_ssq", bufs=2, space="PSUM") as ps_ssq, \
                 tc.tile_pool(name="ps_bc", bufs=2, space="PSUM") as ps_bc, \
                 tc.tile_pool(name="ps_mm", bufs=2, space="PSUM") as ps_mm:
                wq_sb = p1w.tile([128, KB, 512], F32R)
                nc.sync.dma_start(
                    out=wq_sb[:], in_=wq_in.rearrange("(a b) c -> b a c", b=128))
                wkv_sb = p1w.tile([128, KB, 256], F32R)
                nc.sync.dma_start(
                    out=wkv_sb[:], in_=wkv_in.rearrange("(a b) c -> b a c", b=128))
                cosT = p1c.tile([64, S], F32)
                nc.sync.dma_start(out=cosT[:], in_=cos_in[:])
                sinP = p1c.tile([64, S], F32)
                nc.sync.dma_start(out=sinP[:], in_=sin_in[:])
                pmT = p1c.tile([64, 64], F32R)
                nc.sync.dma_start(out=pmT[:], in_=pm_in[:])
                qsr = p1c.tile([1, 64], F32R)
                nc.sync.dma_start(out=qsr[:], in_=qsr_in[:])
                ksr = p1c.tile([1, 64], F32R)
                nc.sync.dma_start(out=ksr[:], in_=ksr_in[:])
                indt = p1c.tile([64, NPAIR, 2, 8], F32R)
                nc.sync.dma_start(
                    out=indt[:], in_=ind_in.rearrange("a h b c -> b a h c"))
                indkt = p1c.tile([64, 2, 2], F32R)
                nc.sync.dma_start(out=indkt[:], in_=indk_in.rearrange("h b c -> b h c"))
                ident = p1c.tile([128, 128], F32R)
                nc.sync.dma_start(out=ident[:], in_=id_in[:])
                epsq = p1c.tile([8, 1], F32)
                nc.any.memset(epsq[:], 64e-6)
                epsk = p1c.tile([2, 1], F32)
                nc.any.memset(epsk[:], 1e-6)

                for s in range(NS1):
                    tok = bass.ts(s, S1)
                    xTs = p1x.tile([128, KB, S1], F32R, name="xTs")
                    nc.sync.dma_start(
                        out=xTs[:],
                        in_=xT_in[:, tok].rearrange("(a b) c -> b a c", b=128))

                    rawAll = [p1t.tile([64, 5, S1], F32, name=f"rawAll{h}", bufs=1)
                              for h in range(2)]
                    sqAll = [p1t.tile([64, 5, S1], F32R, name=f"sqAll{h}", bufs=1)
                             for h in range(2)]
                    raws = [(rawAll[0][:, blk, :], rawAll[1][:, blk, :])
                            for blk in range(5)]
                    sqs = [(sqAll[0][:, blk, :], sqAll[1][:, blk, :])
                           for blk in range(5)]
                    for blk in range(5):
                        ps = ps_proj.tile([128, S1], F32, name="ps")
                        for kb in range(KB):
                            if blk < 4:
                                lhsT = wq_sb[:, kb, bass.ts(blk, 128)]
                            else:
                                lhsT = wkv_sb[:, kb, 0:128]
                            nc.tensor.matmul(ps[:], lhsT, xTs[:, kb, :],
                                             start=(kb == 0), stop=(kb == KB - 1))
                        rawA, rawB = raws[blk]
                        nc.scalar.copy(rawA, ps[0:64, :])
                        nc.vector.tensor_copy(rawB, ps[64:128, :])
                        sqA, sqB = sqs[blk]
                        nc.scalar.square(sqA, rawA)
                        nc.scalar.square(sqB, rawB)

                        if blk == 4:  # v projection: separate accumulator
                            psv = ps_proj.tile([128, S1], F32, name="ps")
                            for kb in range(KB):
                                nc.tensor.matmul(psv[:], wkv_sb[:, kb, 128:256],
                                                 xTs[:, kb, :],
                                                 start=(kb == 0), stop=(kb == KB - 1))
                            vTs = p1t.tile([128, S1], F32R, name="vTs")
                            nc.scalar.copy(vTs[:], psv[:])
                            # transpose to natural V layout [tok, dv]
                            for tb in range(S1 // 128):
                                tbg = (s * S1) // 128 + tb
                                psV = ps_mm.tile([128, 128], F32R, name="psmm")
                                nc.tensor.transpose(
                                    psV[:], vTs[:, bass.ts(tb, 128)], ident[:])
                                nc.vector.tensor_copy(
                                    Vg[0][:, tbg, 0:64], psV[:, 0:64])
                                nc.vector.tensor_copy(
                                    Vg[1][:, tbg, 0:64], psV[:, 64:128])

                    # rms statistics: mean of squares per head
                    psS = ps_ssq.tile([8, S1], F32, name="psS")
                    for m in range(NPAIR):
                        nc.tensor.matmul(psS[:], indt[:, m, 0, :], sqs[m][0],
                                         start=(m == 0), stop=False)
                        nc.tensor.matmul(psS[:], indt[:, m, 1, :], sqs[m][1],
                                         start=False, stop=(m == NPAIR - 1))
                    rq = p1t.tile([8, S1], F32, name="rq")
                    nc.scalar.activation(rq[:], psS[:], AF.Sqrt,
                                         bias=epsq[:], scale=64.0)
                    aq = p1t.tile([8, S1], F32R, name="aq")
                    with nc.allow_low_precision(reason="f32r scale rounding ok"):
                        nc.vector.reciprocal(aq[:], rq[:])
                    aqF = p1t.tile([1, 8 * S1], F32R, name="aqF", bufs=1)
                    nc.sync.dma_start(
                        out=aqF[0:1, :].rearrange("p (a b) -> p a b", a=8),
                        in_=aq[:])

                    psK = ps_ssq.tile([2, S1], F32, name="psS")
                    nc.tensor.matmul(psK[:], indkt[:, 0, :], sqs[4][0],
                                     start=True, stop=False)
                    nc.tensor.matmul(psK[:], indkt[:, 1, :], sqs[4][1],
                                     start=False, stop=True)
                    rk = p1t.tile([2, S1], F32, name="rk")
                    nc.scalar.activation(rk[:], psK[:], AF.Sqrt,
                                         bias=epsk[:], scale=1.0)
                    ak = p1t.tile([2, S1], F32R, name="ak")
                    with nc.allow_low_precision(reason="f32r scale rounding ok"):
                        nc.vector.reciprocal(ak[:], rk[:])
                    akF = p1t.tile([1, 2 * S1], F32R, name="akF", bufs=1)
                    nc.sync.dma_start(
                        out=akF[0:1, :].rearrange("p (a b) -> p a b", a=2),
                        in_=ak[:])

                    # normalize + rope -> qTn/kTn
                    for blk in range(5):
                        if blk < 4:
                            dst, scl, flat, rows = qTn[blk], qsr, aqF, (2 * blk, 2 * blk + 1)
                        else:
                            dst, scl, flat, rows = kTn, ksr, akF, (0, 1)
                        for half in range(2):
                            raw = raws[blk][half]  # [64, S1] slice
                            psBc = ps_bc.tile([64, S1], F32, name="psBc")
                            nc.tensor.matmul(
                                psBc[:], scl[:],
                                flat[0:1, bass.ts(rows[half], S1)],
                                start=True, stop=True)
                            qs1 = p1t.tile([64, S1], F32R, name="qs1")
                            nc.vector.tensor_mul(qs1[:], raw, psBc[:])
                            wv = p1t.tile([64, S1], F32R, name="wv")
                            nc.vector.tensor_mul(wv[:], qs1[:], sinP[:, tok])
                            psR = ps_mm.tile([64, S1], F32, name="psmm")
                            nc.tensor.matmul(psR[:], pmT[:], wv[:],
                                             start=True, stop=True)
                            t2 = p1t.tile([64, S1], F32, name="t2")
                            nc.vector.tensor_mul(t2[:], qs1[:], cosT[:, tok])
                            nc.vector.tensor_add(
                                dst[bass.ts(half, 64), tok], psR[:], t2[:])

            # ================= Phase 2: attention =========================
            with tc.tile_pool(name="p2ctx", bufs=1) as p2ctx, \
                 tc.tile_pool(name="p2wo", bufs=1) as p2wo:
                ctxS = [p2ctx.tile([128, S], F32R, name=f"ctxS{m}")
                        for m in range(NPAIR)]
                wo_sb = p2wo.tile([128, NPAIR, EMB], F32R)
                nc.sync.dma_start(
                    out=wo_sb[:], in_=wo_in.rearrange("(a b) c -> b a c", b=128))

                with tc.tile_pool(name="p2m", bufs=1) as p2m, \
                     tc.tile_pool(name="p2e", bufs=6) as p2e, \
                     tc.tile_pool(name="p2c", bufs=3) as p2c, \
                     tc.tile_pool(name="p2d", bufs=2) as p2d, \
                     tc.tile_pool(name="ps_S", bufs=4, space="PSUM") as ps_S, \
                     tc.tile_pool(name="ps_ctx", bufs=1, space="PSUM") as ps_ctx, \
                     tc.tile_pool(name="ps_dn", bufs=2, space="PSUM") as ps_dn:
                    maskT = p2m.tile([128, 4, S2], F32)
                    nc.sync.dma_start(
                        out=maskT[:], in_=mask_in.rearrange("a b c -> b a c"))

                    for s in range(NS2):
                        qtok = bass.ts(s, S2)
                        nkb = 4 * (s + 1)
                        for m in range(NPAIR):
                            psCtx = [ps_ctx.tile([65, S2], F32, name=f"psCtx{h}")
                                     for h in range(2)]
                            for kb in range(nkb):
                                eS = []
                                for half in range(2):
                                    psSc = ps_S.tile([128, S2], F32, name="psSc")
                                    nc.tensor.matmul(
                                        psSc[:],
                                        kTn[bass.ts(half, 64), bass.ts(kb, 128)],
                                        qTn[m][bass.ts(half, 64), qtok],
                                        start=True, stop=True,
                                        tile_position=(64 * half, 0))
                                    ex = p2e.tile([128, S2], F32R, name="ex")
                                    if kb >= 4 * s:  # diagonal: causal mask
                                        tmp = p2e.tile([128, S2], F32, name="tmp")
                                        nc.vector.tensor_add(
                                            tmp[:], psSc[:],
                                            maskT[:, kb - 4 * s, :])
                                        nc.scalar.activation(ex[:], tmp[:], AF.Exp)
                                    else:
                                        nc.scalar.activation(ex[:], psSc[:], AF.Exp)
                                    eS.append(ex)
                                for half in range(2):
                                    nc.tensor.matmul(
                                        psCtx[half][:],
                                        Vg[half][:, kb, :], eS[half][:],
                                        start=(kb == 0), stop=(kb == nkb - 1))
                            # evict + denominator broadcast + scale
                            dnP = p2d.tile([2, S2], F32, name="dnP")
                            crs = []
                            for half in range(2):
                                cr = p2c.tile([65, S2], F32, name="cr")
                                nc.scalar.copy(cr[:], psCtx[half][:])
                                nc.sync.dma_start(out=dnP[half:half + 1, :],
                                                  in_=cr[64:65, :])
                                crs.append(cr)
                            dnR = p2d.tile([2, S2], F32R, name="dnR")
                            with nc.allow_low_precision(reason="denr"):
                                nc.vector.reciprocal(dnR[:], dnP[:])
                            dnF = p2d.tile([1, 2 * S2], F32R, name="dnF")
                            nc.sync.dma_start(
                                out=dnF[0:1, :].rearrange("p (a b) -> p a b", a=2),
                                in_=dnR[:])
                            for half in range(2):
                                psD = ps_dn.tile([64, S2], F32, name="psD")
                                nc.tensor.matmul(
                                    psD[:], ones64[:],
                                    dnF[0:1, bass.ts(half, S2)],
                                    start=True, stop=True)
                                nc.vector.tensor_mul(
                                    ctxS[m][bass.ts(half, 64), qtok],
                                    crs[half][0:64, :], psD[:])

                # ============= Phase 3: output projection + RS ============
                with tc.tile_pool(name="p3o", bufs=2) as p3o, \
                     tc.tile_pool(name="ps_out", bufs=6, space="PSUM") as ps_out:
                    for c in range(NS2):
                        for ti in range(4):
                            t = 4 * c + ti
                            oSb = p3o.tile([128, EMB], F32, name="oSb")
                            for e in range(4):
                                psO = ps_out.tile([128, 512], F32, name="psO")
                                for m in range(NPAIR):
                                    nc.tensor.matmul(
                                        psO[:],
                                        ctxS[m][:, bass.ts(t, 128)],
                                        wo_sb[:, m, bass.ts(e, 512)],
                                        start=(m == 0), stop=(m == NPAIR - 1))
                                nc.scalar.copy(oSb[:, bass.ts(e, 512)], psO[:])
                            nc.sync.dma_start(
                                out=oPart[bass.ts(t, 128), :], in_=oSb[:])
                        nc.gpsimd.collective_compute(
                            "ReduceScatter", mybir.AluOpType.add,
                            replica_groups=[[0, 1, 2, 3], [4, 5, 6, 7]],
                            ins=[oPart[bass.ts(c, S2), :]],
                            outs=[rsOut[c]])
                        nc.sync.dma_start(out=out_ext[c], in_=rsOut[c])

    nc.finalize()
    return nc


def _host_inputs(x, cos, sin, Wq, Wk, Wv, Wo, q_scale, k_scale):
    f = np.float32
    pm = np.zeros((64, 64), f)
    for k in range(32):
        pm[k, k + 32] = 1.0
    for k in range(32, 64):
        pm[k, k - 32] = -1.0
    cosT = np.ascontiguousarray(cos.T.astype(f))            # [64, S]
    sinT = sin.T.astype(f)
    sinP = np.ascontiguousarray(np.roll(sinT, -32, axis=0))  # s_pre[d]=sin[(d+32)%64]
    ind = np.zeros((NPAIR, 2, 64, 8), f)
    for m in range(NPAIR):
        ind[m, 0, :, 2 * m] = 1.0 / 64
        ind[m, 1, :, 2 * m + 1] = 1.0 / 64
    indk = np.zeros((2, 64, 2), f)
    indk[0, :, 0] = 1.0 / 64
    indk[1, :, 1] = 1.0 / 64
    maskadd = np.zeros((4, 128, S2), f)
    p = np.arange(128)[:, None]
    j = np.arange(S2)[None, :]
    for r in range(4):
        maskadd[r] = np.where(p + 128 * r <= j, 0.0, NEG)
    common = {
        "cosT": cosT, "sinP": sinP, "pm": pm,
        "qsr": np.ascontiguousarray(q_scale.astype(f)[None, :]),
        "ksr": np.ascontiguousarray(k_scale.astype(f)[None, :]),
        "ind": ind, "indk": indk,
        "ones64": np.ones((1, 64), f),
        "vones": np.ones((128, KB), f),
        "maskadd": maskadd,
        "ident": np.eye(128, dtype=f),
    }
    in_maps = []
    for c in range(8):
        b, tp = divmod(c, TP)
        heads = [8 * tp + m for m in range(NPAIR)] + \
                [8 * tp + 4 + m for m in range(NPAIR)]
        order = []
        for m in range(NPAIR):
            order += [heads[m], heads[NPAIR + m]]
        qcols = np.concatenate([np.arange(h * D, (h + 1) * D) for h in order])
        g0, g1 = 2 * tp, 2 * tp + 1
        kvcols = np.concatenate([
            np.arange(g0 * D, (g0 + 1) * D), np.arange(g1 * D, (g1 + 1) * D)])
        im = dict(common)
        im["xT"] = np.ascontiguousarray(x[b].T.astype(f))
        im["wq"] = np.ascontiguousarray(Wq[:, qcols].astype(f))
        im["wkv"] = np.ascontiguousarray(np.concatenate(
            [Wk[:, kvcols], Wv[:, kvcols]], axis=1).astype(f))
        worows = np.concatenate([np.arange(h * D, (h + 1) * D) for h in order])
        im["wo"] = np.ascontiguousarray(Wo[worows, :].astype(f))
        in_maps.append(im)
    return in_maps


def kernel(x, mask, cos, sin, Wq, Wk, Wv, Wo, q_scale, k_scale):
    global LAST_RESULT
    x = np.asarray(x)
    if "nc" not in _CACHE:
        _CACHE["nc"] = _build()
    nc = _CACHE["nc"]
    in_maps = _host_inputs(np.asarray(x), np.asarray(cos), np.asarray(sin),
                           np.asarray(Wq), np.asarray(Wk), np.asarray(Wv),
                           np.asarray(Wo), np.asarray(q_scale),
                           np.asarray(k_scale))
    res = run_bass_kernel_spmd(nc, in_maps, core_ids=list(range(8)))
    LAST_RESULT = res
    out = np.empty((B, S, EMB), np.float32)
    for c in range(8):
        b, tp = divmod(c, TP)
        mo = res.results[c]["myout"]
        for ch in range(NS2):
            r0 = S2 * ch + 128 * tp
            out[b, r0:r0 + 128, :] = mo[ch]
    return out


# revision 6
# speedup vs baseline: 1.0848x; 1.0848x over previous
"""GQA attention block (B=2,S=2048,EMB=2048,H=32,G=8,D=64) on 8 trn2 cores.

Sharding: DP over batch (2) x TP over heads (4). Each core handles one batch
and 8 Q heads / 2 KV groups. Wq/Wk/Wv column-sharded, Wo row-sharded with an
on-device ReduceScatter over each batch's 4-core group; host concatenates the
disjoint row slices.

All heavy matmuls run in float32r (TF32-like, ~11-bit mantissa, full PE rate).
Softmax needs no running max: rmsnorm bounds |logits| <= 8.
"""

import numpy as np

import concourse.bass as bass
import concourse.mybir as mybir
import concourse.tile as tile
from concourse import bacc
from concourse.bass_utils import run_bass_kernel_spmd

F32 = mybir.dt.float32
F32R = mybir.dt.float32r
AF = mybir.ActivationFunctionType

B, S, EMB, H, G, D = 2, 2048, 2048, 32, 8, 64
TP = 4                      # tensor-parallel degree (cores per batch)
HL = H // TP                # 8 local heads
NPAIR = HL // 2             # 4 head pairs (one per 128-feat block)
S1 = 256                    # phase-1 token strip
NS1 = S // S1               # 8
S2 = 512                    # attention query strip
NS2 = S // S2               # 4
KB = S // 128               # 16 key blocks
NEG = -30.0

_CACHE = {}
LAST_RESULT = None


def _build():
    nc = bacc.Bacc(None, target_bir_lowering=False, debug=False)

    xT_in = nc.dram_tensor("xT", [EMB, S], F32R, kind="ExternalInput")
    wq_in = nc.dram_tensor("wq", [EMB, 512], F32R, kind="ExternalInput")
    wkv_in = nc.dram_tensor("wkv", [EMB, 256], F32R, kind="ExternalInput")
    wo_in = nc.dram_tensor("wo", [512, EMB], F32R, kind="ExternalInput")
    cos_in = nc.dram_tensor("cosT", [64, S], F32, kind="ExternalInput")
    sin_in = nc.dram_tensor("sinP", [64, S], F32, kind="ExternalInput")
    pm_in = nc.dram_tensor("pm", [64, 64], F32R, kind="ExternalInput")
    qsr_in = nc.dram_tensor("qsr", [1, 64], F32R, kind="ExternalInput")
    ksr_in = nc.dram_tensor("ksr", [1, 64], F32R, kind="ExternalInput")
    ind_in = nc.dram_tensor("ind", [NPAIR, 2, 64, 8], F32R, kind="ExternalInput")
    indk_in = nc.dram_tensor("indk", [2, 64, 2], F32R, kind="ExternalInput")
    ones_in = nc.dram_tensor("ones64", [1, 64], F32R, kind="ExternalInput")
    vones_in = nc.dram_tensor("vones", [128, KB], F32R, kind="ExternalInput")
    mask_in = nc.dram_tensor("maskadd", [4, 128, S2], F32, kind="ExternalInput")
    id_in = nc.dram_tensor("ident", [128, 128], F32R, kind="ExternalInput")
    out_ext = nc.dram_tensor("myout", [NS2, 128, EMB], F32, kind="ExternalOutput")

    with tile.TileContext(nc) as tc:
        # ---------- long-lived tensors ----------
        with tc.tile_pool(name="resident", bufs=1) as pres, \
             tc.tile_pool(name="dram", bufs=1, space="DRAM") as pdram:
            qTn = [pres.tile([128, S], F32R, name=f"qTn{m}") for m in range(NPAIR)]
            kTn = pres.tile([128, S], F32R)
            Vg = [pres.tile([128, KB, 65], F32R, name=f"Vg{g}") for g in range(2)]
            ones64 = pres.tile([1, 64], F32R)
            nc.sync.dma_start(out=ones64[:], in_=ones_in[:])
            for g in range(2):
                nc.sync.dma_start(out=Vg[g][:, :, 64:65], in_=vones_in[:, :, None])
            oPart = pdram.tile([S, EMB], F32)
            rsOut = pdram.tile([NS2, 128, EMB], F32)

            # ================= Phase 1: projections + norm + rope ========
            with tc.tile_pool(name="p1w", bufs=1) as p1w, \
                 tc.tile_pool(name="p1x", bufs=2) as p1x, \
                 tc.tile_pool(name="p1c", bufs=1) as p1c, \
                 tc.tile_pool(name="p1t", bufs=2) as p1t, \
                 tc.tile_pool(name="ps_proj", bufs=2, space="PSUM") as ps_proj, \
                 tc.tile_pool(name="ps_ssq", bufs=2, space="PSUM") as ps_ssq, \
                 tc.tile_pool(name="ps_bc", bufs=2, space="PSUM") as ps_bc, \
                 tc.tile_pool(name="ps_mm", bufs=2, space="PSUM") as ps_mm:
                wq_sb = p1w.tile([128, KB, 512], F32R)
                nc.sync.dma_start(
                    out=wq_sb[:], in_=wq_in.rearrange("(a b) c -> b a c", b=128))
                wkv_sb = p1w.tile([128, KB, 256], F32R)
                nc.sync.dma_start(
                    out=wkv_sb[:], in_=wkv_in.rearrange("(a b) c -> b a c", b=128))
                cosT = p1c.tile([64, S], F32)
                nc.sync.dma_start(out=cosT[:], in_=cos_in[:])
                sinP = p1c.tile([64, S], F32)
                nc.sync.dma_start(out=sinP[:], in_=sin_in[:])
                pmT = p1c.tile([64, 64], F32R)
                nc.sync.dma_start(out=pmT[:], in_=pm_in[:])
                qsr = p1c.tile([1, 64], F32R)
                nc.sync.dma_start(out=qsr[:], in_=qsr_in[:])
                ksr = p1c.tile([1, 64], F32R)
                nc.sync.dma_start(out=ksr[:], in_=ksr_in[:])
                indt = p1c.tile([64, NPAIR, 2, 8], F32R)
                nc.sync.dma_start(
                    out=indt[:], in_=ind_in.rearrange("a h b c -> b a h c"))
                indkt = p1c.tile([64, 2, 2], F32R)
                nc.sync.dma_start(out=indkt[:], in_=indk_in.rearrange("h b c -> b h c"))
                ident = p1c.tile([128, 128], F32R)
                nc.sync.dma_start(out=ident[:], in_=id_in[:])
                epsq = p1c.tile([8, 1], F32)
                nc.any.memset(epsq[:], 64e-6)
                epsk = p1c.tile([2, 1], F32)
                nc.any.memset(epsk[:], 1e-6)

                for s in range(NS1):
                    tok = bass.ts(s, S1)
                    xTs = p1x.tile([128, KB, S1], F32R, name="xTs")
                    nc.sync.dma_start(
                        out=xTs[:],
                        in_=xT_in[:, tok].rearrange("(a b) c -> b a c", b=128))

                    rawAll = [p1t.tile([64, 5, S1], F32, name=f"rawAll{h}", bufs=1)
                              for h in range(2)]
                    sqAll = [p1t.tile([64, 5, S1], F32R, name=f"sqAll{h}", bufs=1)
                             for h in range(2)]
                    raws = [(rawAll[0][:, blk, :], rawAll[1][:, blk, :])
                            for blk in range(5)]
                    sqs = [(sqAll[0][:, blk, :], sqAll[1][:, blk, :])
                           for blk in range(5)]
                    for blk in range(5):
                        ps = ps_proj.tile([128, S1], F32, name="ps")
                        for kb in range(KB):
                            if blk < 4:
                                lhsT = wq_sb[:, kb, bass.ts(blk, 128)]
                            else:
                                lhsT = wkv_sb[:, kb, 0:128]
                            nc.tensor.matmul(ps[:], lhsT, xTs[:, kb, :],
                                             start=(kb == 0), stop=(kb == KB - 1))
                        rawA, rawB = raws[blk]
                        nc.scalar.copy(rawA, ps[0:64, :])
                        nc.vector.tensor_copy(rawB, ps[64:128, :])
                        sqA, sqB = sqs[blk]
                        nc.scalar.square(sqA, rawA)
                        nc.scalar.square(sqB, rawB)

                        if blk == 4:  # v projection: separate accumulator
                            psv = ps_proj.tile([128, S1], F32, name="ps")
                            for kb in range(KB):
                                nc.tensor.matmul(psv[:], wkv_sb[:, kb, 128:256],
                                                 xTs[:, kb, :],
                                                 start=(kb == 0), stop=(kb == KB - 1))
                            vTs = p1t.tile([128, S1], F32R, name="vTs")
                            nc.scalar.copy(vTs[:], psv[:])
                            # transpose to natural V layout [tok, dv]
                            for tb in range(S1 // 128):
                                tbg = (s * S1) // 128 + tb
                                psV = ps_mm.tile([128, 128], F32R, name="psmm")
                                nc.tensor.transpose(
                                    psV[:], vTs[:, bass.ts(tb, 128)], ident[:])
                                nc.vector.tensor_copy(
                                    Vg[0][:, tbg, 0:64], psV[:, 0:64])
                                nc.vector.tensor_copy(
                                    Vg[1][:, tbg, 0:64], psV[:, 64:128])

                    # rms statistics: mean of squares per head
                    psS = ps_ssq.tile([8, S1], F32, name="psS")
                    for m in range(NPAIR):
                        nc.tensor.matmul(psS[:], indt[:, m, 0, :], sqs[m][0],
                                         start=(m == 0), stop=False)
                        nc.tensor.matmul(psS[:], indt[:, m, 1, :], sqs[m][1],
                                         start=False, stop=(m == NPAIR - 1))
                    rq = p1t.tile([8, S1], F32, name="rq")
                    nc.scalar.activation(rq[:], psS[:], AF.Sqrt,
                                         bias=epsq[:], scale=64.0)
                    aq = p1t.tile([8, S1], F32R, name="aq")
                    with nc.allow_low_precision(reason="f32r scale rounding ok"):
                        nc.vector.reciprocal(aq[:], rq[:])
                    aqF = p1t.tile([1, 8 * S1], F32R, name="aqF", bufs=1)
                    nc.sync.dma_start(
                        out=aqF[0:1, :].rearrange("p (a b) -> p a b", a=8),
                        in_=aq[:])

                    psK = ps_ssq.tile([2, S1], F32, name="psS")
                    nc.tensor.matmul(psK[:], indkt[:, 0, :], sqs[4][0],
                                     start=True, stop=False)
                    nc.tensor.matmul(psK[:], indkt[:, 1, :], sqs[4][1],
                                     start=False, stop=True)
                    rk = p1t.tile([2, S1], F32, name="rk")
                    nc.scalar.activation(rk[:], psK[:], AF.Sqrt,
                                         bias=epsk[:], scale=1.0)
                    ak = p1t.tile([2, S1], F32R, name="ak")
                    with nc.allow_low_precision(reason="f32r scale rounding ok"):
                        nc.vector.reciprocal(ak[:], rk[:])
                    akF = p1t.tile([1, 2 * S1], F32R, name="akF", bufs=1)
                    nc.sync.dma_start(
                        out=akF[0:1, :].rearrange("p (a b) -> p a b", a=2),
                        in_=ak[:])

                    # normalize + rope -> qTn/kTn
                    for blk in range(5):
                        if blk < 4:
                            dst, scl, flat, rows = qTn[blk], qsr, aqF, (2 * blk, 2 * blk + 1)
                        else:
                            dst, scl, flat, rows = kTn, ksr, akF, (0, 1)
                        for half in range(2):
                            raw = raws[blk][half]  # [64, S1] slice
                            psBc = ps_bc.tile([64, S1], F32, name="psBc")
                            nc.tensor.matmul(
                                psBc[:], scl[:],
                                flat[0:1, bass.ts(rows[half], S1)],
                                start=True, stop=True)
                            qs1 = p1t.tile([64, S1], F32R, name="qs1")
                            nc.vector.tensor_mul(qs1[:], raw, psBc[:])
                            wv = p1t.tile([64, S1], F32R, name="wv")
                            nc.vector.tensor_mul(wv[:], qs1[:], sinP[:, tok])
                            psR = ps_mm.tile([64, S1], F32, name="psmm")
                            nc.tensor.matmul(psR[:], pmT[:], wv[:],
                                             start=True, stop=True)
                            t2 = p1t.tile([64, S1], F32, name="t2")
                            nc.vector.tensor_mul(t2[:], qs1[:], cosT[:, tok])
                            nc.vector.tensor_add(
                                dst[bass.ts(half, 64), tok], psR[:], t2[:])

            # ================= Phase 2: attention =========================
            with tc.tile_pool(name="p2ctx", bufs=1) as p2ctx, \
                 tc.tile_pool(name="p2wo", bufs=1) as p2wo:
                ctxS = [p2ctx.tile([128, S], F32R, name=f"ctxS{m}")
                        for m in range(NPAIR)]
                wo_sb = p2wo.tile([128, NPAIR, EMB], F32R)
                nc.sync.dma_start(
                    out=wo_sb[:], in_=wo_in.rearrange("(a b) c -> b a c", b=128))

                with tc.tile_pool(name="p2m", bufs=1) as p2m, \
                     tc.tile_pool(name="p2e", bufs=6) as p2e, \
                     tc.tile_pool(name="p2c", bufs=3) as p2c, \
                     tc.tile_pool(name="p2d", bufs=2) as p2d, \
                     tc.tile_pool(name="ps_S", bufs=2, space="PSUM") as ps_S, \
                     tc.tile_pool(name="ps_ctx", bufs=1, space="PSUM") as ps_ctx, \
                     tc.tile_pool(name="ps_dn", bufs=2, space="PSUM") as ps_dn, \
                 tc.tile_pool(name="p3o", bufs=2) as p3o, \
                 tc.tile_pool(name="ps_out", bufs=2, space="PSUM") as ps_out:
                    maskT = p2m.tile([128, 4, S2], F32)
                    nc.sync.dma_start(
                        out=maskT[:], in_=mask_in.rearrange("a b c -> b a c"))

                    for s in range(NS2):
                        qtok = bass.ts(s, S2)
                        nkb = 4 * (s + 1)
                        for m in range(NPAIR):
                            psCtx = [ps_ctx.tile([65, S2], F32, name=f"psCtx{h}")
                                     for h in range(2)]
                            for kb in range(nkb):
                                eS = []
                                for half in range(2):
                                    psSc = ps_S.tile([128, S2], F32, name="psSc")
                                    nc.tensor.matmul(
                                        psSc[:],
                                        kTn[bass.ts(half, 64), bass.ts(kb, 128)],
                                        qTn[m][bass.ts(half, 64), qtok],
                                        start=True, stop=True,
                                        tile_position=(64 * half, 0))
                                    ex = p2e.tile([128, S2], F32R, name="ex")
                                    if kb >= 4 * s:  # diagonal: causal mask
                                        tmp = p2e.tile([128, S2], F32, name="tmp")
                                        nc.vector.tensor_add(
                                            tmp[:], psSc[:],
                                            maskT[:, kb - 4 * s, :])
                                        nc.scalar.activation(ex[:], tmp[:], AF.Exp)
                                    else:
                                        nc.scalar.activation(ex[:], psSc[:], AF.Exp)
                                    eS.append(ex)
                                for half in range(2):
                                    nc.tensor.matmul(
                                        psCtx[half][:],
                                        Vg[half][:, kb, :], eS[half][:],
                                        start=(kb == 0), stop=(kb == nkb - 1))
                            # evict + denominator broadcast + scale
                            dnP = p2d.tile([2, S2], F32, name="dnP")
                            crs = []
                            for half in range(2):
                                cr = p2c.tile([65, S2], F32, name="cr")
                                nc.scalar.copy(cr[:], psCtx[half][:])
                                nc.sync.dma_start(out=dnP[half:half + 1, :],
                                                  in_=cr[64:65, :])
                                crs.append(cr)
                            dnR = p2d.tile([2, S2], F32R, name="dnR")
                            with nc.allow_low_precision(reason="denr"):
                                nc.vector.reciprocal(dnR[:], dnP[:])
                            dnF = p2d.tile([1, 2 * S2], F32R, name="dnF")
                            nc.sync.dma_start(
                                out=dnF[0:1, :].rearrange("p (a b) -> p a b", a=2),
                                in_=dnR[:])
                            for half in range(2):
                                psD = ps_dn.tile([64, S2], F32, name="psD")
                                nc.tensor.matmul(
                                    psD[:], ones64[:],
                                    dnF[0:1, bass.ts(half, S2)],
                                    start=True, stop=True)
                                nc.vector.tensor_mul(
                                    ctxS[m][bass.ts(half, 64), qtok],
                                    crs[half][0:64, :], psD[:])

                        # ---- output projection + RS for this chunk ----
                        for ti in range(4):
                            t = 4 * s + ti
                            oSb = p3o.tile([128, EMB], F32, name="oSb")
                            for e in range(4):
                                psO = ps_out.tile([128, 512], F32, name="psO")
                                for m in range(NPAIR):
                                    nc.tensor.matmul(
                                        psO[:],
                                        ctxS[m][:, bass.ts(t, 128)],
                                        wo_sb[:, m, bass.ts(e, 512)],
                                        start=(m == 0), stop=(m == NPAIR - 1))
                                nc.scalar.copy(oSb[:, bass.ts(e, 512)], psO[:])
                            nc.sync.dma_start(
                                out=oPart[bass.ts(t, 128), :], in_=oSb[:])
                        nc.gpsimd.collective_compute(
                            "ReduceScatter", mybir.AluOpType.add,
                            replica_groups=[[0, 1, 2, 3], [4, 5, 6, 7]],
                            ins=[oPart[bass.ts(s, S2), :]],
                            outs=[rsOut[s]])
                        nc.sync.dma_start(out=out_ext[s], in_=rsOut[s])

    nc.finalize()
    return nc


def _host_inputs(x, cos, sin, Wq, Wk, Wv, Wo, q_scale, k_scale):
    f = np.float32
    pm = np.zeros((64, 64), f)
    for k in range(32):
        pm[k, k + 32] = 1.0
    for k in range(32, 64):
        pm[k, k - 32] = -1.0
    cosT = np.ascontiguousarray(cos.T.astype(f))            # [64, S]
    sinT = sin.T.astype(f)
    sinP = np.ascontiguousarray(np.roll(sinT, -32, axis=0))  # s_pre[d]=sin[(d+32)%64]
    ind = np.zeros((NPAIR, 2, 64, 8), f)
    for m in range(NPAIR):
        ind[m, 0, :, 2 * m] = 1.0 / 64
        ind[m, 1, :, 2 * m + 1] = 1.0 / 64
    indk = np.zeros((2, 64, 2), f)
    indk[0, :, 0] = 1.0 / 64
    indk[1, :, 1] = 1.0 / 64
    maskadd = np.zeros((4, 128, S2), f)
    p = np.arange(128)[:, None]
    j = np.arange(S2)[None, :]
    for r in range(4):
        maskadd[r] = np.where(p + 128 * r <= j, 0.0, NEG)
    common = {
        "cosT": cosT, "sinP": sinP, "pm": pm,
        "qsr": np.ascontiguousarray(q_scale.astype(f)[None, :]),
        "ksr": np.ascontiguousarray(k_scale.astype(f)[None, :]),
        "ind": ind, "indk": indk,
        "ones64": np.ones((1, 64), f),
        "vones": np.ones((128, KB), f),
        "maskadd": maskadd,
        "ident": np.eye(128, dtype=f),
    }
    in_maps = []
    for c in range(8):
        b, tp = divmod(c, TP)
        heads = [8 * tp + m for m in range(NPAIR)] + \
                [8 * tp + 4 + m for m in range(NPAIR)]
        order = []
        for m in range(NPAIR):
            order += [heads[m], heads[NPAIR + m]]
        qcols = np.concatenate([np.arange(h * D, (h + 1) * D) for h in order])
        g0, g1 = 2 * tp, 2 * tp + 1
        kvcols = np.concatenate([
            np.arange(g0 * D, (g0 + 1) * D), np.arange(g1 * D, (g1 + 1) * D)])
        im = dict(common)
        im["xT"] = np.ascontiguousarray(x[b].T.astype(f))
        im["wq"] = np.ascontiguousarray(Wq[:, qcols].astype(f))
        im["wkv"] = np.ascontiguousarray(np.concatenate(
            [Wk[:, kvcols], Wv[:, kvcols]], axis=1).astype(f))
        worows = np.concatenate([np.arange(h * D, (h + 1) * D) for h in order])
        im["wo"] = np.ascontiguousarray(Wo[worows, :].astype(f))
        in_maps.append(im)
    return in_maps


def kernel(x, mask, cos, sin, Wq, Wk, Wv, Wo, q_scale, k_scale):
    global LAST_RESULT
    x = np.asarray(x)
    if "nc" not in _CACHE:
        _CACHE["nc"] = _build()
    nc = _CACHE["nc"]
    in_maps = _host_inputs(np.asarray(x), np.asarray(cos), np.asarray(sin),
                           np.asarray(Wq), np.asarray(Wk), np.asarray(Wv),
                           np.asarray(Wo), np.asarray(q_scale),
                           np.asarray(k_scale))
    res = run_bass_kernel_spmd(nc, in_maps, core_ids=list(range(8)))
    LAST_RESULT = res
    out = np.empty((B, S, EMB), np.float32)
    for c in range(8):
        b, tp = divmod(c, TP)
        mo = res.results[c]["myout"]
        for ch in range(NS2):
            r0 = S2 * ch + 128 * tp
            out[b, r0:r0 + 128, :] = mo[ch]
    return out


# revision 7
# speedup vs baseline: 1.1607x; 1.0700x over previous
"""GQA attention block (B=2,S=2048,EMB=2048,H=32,G=8,D=64) on 8 trn2 cores.

Sharding: DP over batch (2) x TP over heads (4). Each core handles one batch
and 8 Q heads / 2 KV groups. Wq/Wk/Wv column-sharded, Wo row-sharded with an
on-device ReduceScatter over each batch's 4-core group; host concatenates the
disjoint row slices.

All heavy matmuls run in float32r (TF32-like, ~11-bit mantissa, full PE rate).
Softmax needs no running max: rmsnorm bounds |logits| <= 8.
"""

import numpy as np

import concourse.bass as bass
import concourse.mybir as mybir
import concourse.tile as tile
from concourse import bacc
from concourse.bass_utils import run_bass_kernel_spmd

F32 = mybir.dt.float32
F32R = mybir.dt.float32r
AF = mybir.ActivationFunctionType

B, S, EMB, H, G, D = 2, 2048, 2048, 32, 8, 64
TP = 4                      # tensor-parallel degree (cores per batch)
HL = H // TP                # 8 local heads
NPAIR = HL // 2             # 4 head pairs (one per 128-feat block)
S1 = 256                    # phase-1 token strip
NS1 = S // S1               # 8
S2 = 512                    # attention query strip
NS2 = S // S2               # 4
KB = S // 128               # 16 key blocks
NEG = -30.0

_CACHE = {}
LAST_RESULT = None


def _build():
    nc = bacc.Bacc(None, target_bir_lowering=False, debug=False)

    xT_in = nc.dram_tensor("xT", [EMB, S], F32R, kind="ExternalInput")
    wq_in = nc.dram_tensor("wq", [EMB, 512], F32R, kind="ExternalInput")
    wkv_in = nc.dram_tensor("wkv", [EMB, 256], F32R, kind="ExternalInput")
    wo_in = nc.dram_tensor("wo", [512, EMB], F32R, kind="ExternalInput")
    cos_in = nc.dram_tensor("cosT", [64, S], F32, kind="ExternalInput")
    sin_in = nc.dram_tensor("sinP", [64, S], F32, kind="ExternalInput")
    pm_in = nc.dram_tensor("pm", [64, 64], F32R, kind="ExternalInput")
    qsr_in = nc.dram_tensor("qsr", [1, 64], F32R, kind="ExternalInput")
    ksr_in = nc.dram_tensor("ksr", [1, 64], F32R, kind="ExternalInput")
    ind_in = nc.dram_tensor("ind", [NPAIR, 2, 64, 8], F32R, kind="ExternalInput")
    indk_in = nc.dram_tensor("indk", [2, 64, 2], F32R, kind="ExternalInput")
    ones_in = nc.dram_tensor("ones64", [1, 64], F32R, kind="ExternalInput")
    vones_in = nc.dram_tensor("vones", [128, KB], F32R, kind="ExternalInput")
    mask_in = nc.dram_tensor("maskadd", [4, 128, S2], F32, kind="ExternalInput")
    id_in = nc.dram_tensor("ident", [128, 128], F32R, kind="ExternalInput")
    out_ext = nc.dram_tensor("myout", [NS2, 128, EMB], mybir.dt.bfloat16, kind="ExternalOutput")

    with tile.TileContext(nc) as tc:
        # ---------- long-lived tensors ----------
        with tc.tile_pool(name="resident", bufs=1) as pres, \
             tc.tile_pool(name="dram", bufs=1, space="DRAM") as pdram:
            qTn = [pres.tile([128, S], F32R, name=f"qTn{m}") for m in range(NPAIR)]
            kTn = pres.tile([128, S], F32R)
            Vg = [pres.tile([128, KB, 65], F32R, name=f"Vg{g}") for g in range(2)]
            ones64 = pres.tile([1, 64], F32R)
            nc.sync.dma_start(out=ones64[:], in_=ones_in[:])
            for g in range(2):
                nc.sync.dma_start(out=Vg[g][:, :, 64:65], in_=vones_in[:, :, None])
            oPart = pdram.tile([S, EMB], mybir.dt.bfloat16)
            rsOut = pdram.tile([NS2, 128, EMB], mybir.dt.bfloat16)

            # ================= Phase 1: projections + norm + rope ========
            with tc.tile_pool(name="p1w", bufs=1) as p1w, \
                 tc.tile_pool(name="p1x", bufs=2) as p1x, \
                 tc.tile_pool(name="p1c", bufs=1) as p1c, \
                 tc.tile_pool(name="p1t", bufs=2) as p1t, \
                 tc.tile_pool(name="ps_proj", bufs=2, space="PSUM") as ps_proj, \
                 tc.tile_pool(name="ps_ssq", bufs=2, space="PSUM") as ps_ssq, \
                 tc.tile_pool(name="ps_bc", bufs=2, space="PSUM") as ps_bc, \
                 tc.tile_pool(name="ps_mm", bufs=2, space="PSUM") as ps_mm:
                wq_sb = p1w.tile([128, KB, 512], F32R)
                nc.sync.dma_start(
                    out=wq_sb[:], in_=wq_in.rearrange("(a b) c -> b a c", b=128))
                wkv_sb = p1w.tile([128, KB, 256], F32R)
                nc.sync.dma_start(
                    out=wkv_sb[:], in_=wkv_in.rearrange("(a b) c -> b a c", b=128))
                cosT = p1c.tile([64, S], F32)
                nc.sync.dma_start(out=cosT[:], in_=cos_in[:])
                sinP = p1c.tile([64, S], F32)
                nc.sync.dma_start(out=sinP[:], in_=sin_in[:])
                pmT = p1c.tile([64, 64], F32R)
                nc.sync.dma_start(out=pmT[:], in_=pm_in[:])
                qsr = p1c.tile([1, 64], F32R)
                nc.sync.dma_start(out=qsr[:], in_=qsr_in[:])
                ksr = p1c.tile([1, 64], F32R)
                nc.sync.dma_start(out=ksr[:], in_=ksr_in[:])
                indt = p1c.tile([64, NPAIR, 2, 8], F32R)
                nc.sync.dma_start(
                    out=indt[:], in_=ind_in.rearrange("a h b c -> b a h c"))
                indkt = p1c.tile([64, 2, 2], F32R)
                nc.sync.dma_start(out=indkt[:], in_=indk_in.rearrange("h b c -> b h c"))
                ident = p1c.tile([128, 128], F32R)
                nc.sync.dma_start(out=ident[:], in_=id_in[:])
                epsq = p1c.tile([8, 1], F32)
                nc.any.memset(epsq[:], 64e-6)
                epsk = p1c.tile([2, 1], F32)
                nc.any.memset(epsk[:], 1e-6)

                for s in range(NS1):
                    tok = bass.ts(s, S1)
                    xTs = p1x.tile([128, KB, S1], F32R, name="xTs")
                    nc.sync.dma_start(
                        out=xTs[:],
                        in_=xT_in[:, tok].rearrange("(a b) c -> b a c", b=128))

                    rawAll = [p1t.tile([64, 5, S1], F32, name=f"rawAll{h}", bufs=1)
                              for h in range(2)]
                    sqAll = [p1t.tile([64, 5, S1], F32R, name=f"sqAll{h}", bufs=1)
                             for h in range(2)]
                    raws = [(rawAll[0][:, blk, :], rawAll[1][:, blk, :])
                            for blk in range(5)]
                    sqs = [(sqAll[0][:, blk, :], sqAll[1][:, blk, :])
                           for blk in range(5)]
                    for blk in range(5):
                        ps = ps_proj.tile([128, S1], F32, name="ps")
                        for kb in range(KB):
                            if blk < 4:
                                lhsT = wq_sb[:, kb, bass.ts(blk, 128)]
                            else:
                                lhsT = wkv_sb[:, kb, 0:128]
                            nc.tensor.matmul(ps[:], lhsT, xTs[:, kb, :],
                                             start=(kb == 0), stop=(kb == KB - 1))
                        rawA, rawB = raws[blk]
                        nc.scalar.copy(rawA, ps[0:64, :])
                        nc.vector.tensor_copy(rawB, ps[64:128, :])
                        sqA, sqB = sqs[blk]
                        nc.scalar.square(sqA, rawA)
                        nc.scalar.square(sqB, rawB)

                        if blk == 4:  # v projection: separate accumulator
                            psv = ps_proj.tile([128, S1], F32, name="ps")
                            for kb in range(KB):
                                nc.tensor.matmul(psv[:], wkv_sb[:, kb, 128:256],
                                                 xTs[:, kb, :],
                                                 start=(kb == 0), stop=(kb == KB - 1))
                            vTs = p1t.tile([128, S1], F32R, name="vTs")
                            nc.scalar.copy(vTs[:], psv[:])
                            # transpose to natural V layout [tok, dv]
                            for tb in range(S1 // 128):
                                tbg = (s * S1) // 128 + tb
                                psV = ps_mm.tile([128, 128], F32R, name="psmm")
                                nc.tensor.transpose(
                                    psV[:], vTs[:, bass.ts(tb, 128)], ident[:])
                                nc.vector.tensor_copy(
                                    Vg[0][:, tbg, 0:64], psV[:, 0:64])
                                nc.vector.tensor_copy(
                                    Vg[1][:, tbg, 0:64], psV[:, 64:128])

                    # rms statistics: mean of squares per head
                    psS = ps_ssq.tile([8, S1], F32, name="psS")
                    for m in range(NPAIR):
                        nc.tensor.matmul(psS[:], indt[:, m, 0, :], sqs[m][0],
                                         start=(m == 0), stop=False)
                        nc.tensor.matmul(psS[:], indt[:, m, 1, :], sqs[m][1],
                                         start=False, stop=(m == NPAIR - 1))
                    rq = p1t.tile([8, S1], F32, name="rq")
                    nc.scalar.activation(rq[:], psS[:], AF.Sqrt,
                                         bias=epsq[:], scale=64.0)
                    aq = p1t.tile([8, S1], F32R, name="aq")
                    with nc.allow_low_precision(reason="f32r scale rounding ok"):
                        nc.vector.reciprocal(aq[:], rq[:])
                    aqF = p1t.tile([1, 8 * S1], F32R, name="aqF", bufs=1)
                    nc.sync.dma_start(
                        out=aqF[0:1, :].rearrange("p (a b) -> p a b", a=8),
                        in_=aq[:])

                    psK = ps_ssq.tile([2, S1], F32, name="psS")
                    nc.tensor.matmul(psK[:], indkt[:, 0, :], sqs[4][0],
                                     start=True, stop=False)
                    nc.tensor.matmul(psK[:], indkt[:, 1, :], sqs[4][1],
                                     start=False, stop=True)
                    rk = p1t.tile([2, S1], F32, name="rk")
                    nc.scalar.activation(rk[:], psK[:], AF.Sqrt,
                                         bias=epsk[:], scale=1.0)
                    ak = p1t.tile([2, S1], F32R, name="ak")
                    with nc.allow_low_precision(reason="f32r scale rounding ok"):
                        nc.vector.reciprocal(ak[:], rk[:])
                    akF = p1t.tile([1, 2 * S1], F32R, name="akF", bufs=1)
                    nc.sync.dma_start(
                        out=akF[0:1, :].rearrange("p (a b) -> p a b", a=2),
                        in_=ak[:])

                    # normalize + rope -> qTn/kTn
                    for blk in range(5):
                        if blk < 4:
                            dst, scl, flat, rows = qTn[blk], qsr, aqF, (2 * blk, 2 * blk + 1)
                        else:
                            dst, scl, flat, rows = kTn, ksr, akF, (0, 1)
                        for half in range(2):
                            raw = raws[blk][half]  # [64, S1] slice
                            psBc = ps_bc.tile([64, S1], F32, name="psBc")
                            nc.tensor.matmul(
                                psBc[:], scl[:],
                                flat[0:1, bass.ts(rows[half], S1)],
                                start=True, stop=True)
                            qs1 = p1t.tile([64, S1], F32R, name="qs1")
                            nc.vector.tensor_mul(qs1[:], raw, psBc[:])
                            wv = p1t.tile([64, S1], F32R, name="wv")
                            nc.vector.tensor_mul(wv[:], qs1[:], sinP[:, tok])
                            psR = ps_mm.tile([64, S1], F32, name="psmm")
                            nc.tensor.matmul(psR[:], pmT[:], wv[:],
                                             start=True, stop=True)
                            t2 = p1t.tile([64, S1], F32, name="t2")
                            nc.vector.tensor_mul(t2[:], qs1[:], cosT[:, tok])
                            nc.vector.tensor_add(
                                dst[bass.ts(half, 64), tok], psR[:], t2[:])

            # ================= Phase 2: attention =========================
            with tc.tile_pool(name="p2ctx", bufs=1) as p2ctx, \
                 tc.tile_pool(name="p2wo", bufs=1) as p2wo:
                ctxS = [p2ctx.tile([128, S], F32R, name=f"ctxS{m}")
                        for m in range(NPAIR)]
                wo_sb = p2wo.tile([128, NPAIR, EMB], F32R)
                nc.sync.dma_start(
                    out=wo_sb[:], in_=wo_in.rearrange("(a b) c -> b a c", b=128))

                with tc.tile_pool(name="p2m", bufs=1) as p2m, \
                     tc.tile_pool(name="p2e", bufs=6) as p2e, \
                     tc.tile_pool(name="p2c", bufs=3) as p2c, \
                     tc.tile_pool(name="p2d", bufs=2) as p2d, \
                     tc.tile_pool(name="ps_S", bufs=2, space="PSUM") as ps_S, \
                     tc.tile_pool(name="ps_ctx", bufs=1, space="PSUM") as ps_ctx, \
                     tc.tile_pool(name="ps_dn", bufs=2, space="PSUM") as ps_dn, \
                 tc.tile_pool(name="p3o", bufs=2) as p3o, \
                 tc.tile_pool(name="ps_out", bufs=2, space="PSUM") as ps_out:
                    maskT = p2m.tile([128, 4, S2], F32)
                    nc.sync.dma_start(
                        out=maskT[:], in_=mask_in.rearrange("a b c -> b a c"))

                    for s in range(NS2):
                        qtok = bass.ts(s, S2)
                        nkb = 4 * (s + 1)
                        for m in range(NPAIR):
                            psCtx = [ps_ctx.tile([65, S2], F32, name=f"psCtx{h}")
                                     for h in range(2)]
                            for kb in range(nkb):
                                eS = []
                                for half in range(2):
                                    psSc = ps_S.tile([128, S2], F32, name="psSc")
                                    nc.tensor.matmul(
                                        psSc[:],
                                        kTn[bass.ts(half, 64), bass.ts(kb, 128)],
                                        qTn[m][bass.ts(half, 64), qtok],
                                        start=True, stop=True,
                                        tile_position=(64 * half, 0))
                                    ex = p2e.tile([128, S2], F32R, name="ex")
                                    if kb >= 4 * s:  # diagonal: causal mask
                                        tmp = p2e.tile([128, S2], F32, name="tmp")
                                        nc.vector.tensor_add(
                                            tmp[:], psSc[:],
                                            maskT[:, kb - 4 * s, :])
                                        nc.scalar.activation(ex[:], tmp[:], AF.Exp)
                                    else:
                                        nc.scalar.activation(ex[:], psSc[:], AF.Exp)
                                    eS.append(ex)
                                for half in range(2):
                                    nc.tensor.matmul(
                                        psCtx[half][:],
                                        Vg[half][:, kb, :], eS[half][:],
                                        start=(kb == 0), stop=(kb == nkb - 1))
                            # evict + denominator broadcast + scale
                            dnP = p2d.tile([2, S2], F32, name="dnP")
                            crs = []
                            for half in range(2):
                                cr = p2c.tile([65, S2], F32, name="cr")
                                nc.scalar.copy(cr[:], psCtx[half][:])
                                nc.sync.dma_start(out=dnP[half:half + 1, :],
                                                  in_=cr[64:65, :])
                                crs.append(cr)
                            dnR = p2d.tile([2, S2], F32R, name="dnR")
                            with nc.allow_low_precision(reason="denr"):
                                nc.vector.reciprocal(dnR[:], dnP[:])
                            dnF = p2d.tile([1, 2 * S2], F32R, name="dnF")
                            nc.sync.dma_start(
                                out=dnF[0:1, :].rearrange("p (a b) -> p a b", a=2),
                                in_=dnR[:])
                            for half in range(2):
                                psD = ps_dn.tile([64, S2], F32, name="psD")
                                nc.tensor.matmul(
                                    psD[:], ones64[:],
                                    dnF[0:1, bass.ts(half, S2)],
                                    start=True, stop=True)
                                nc.vector.tensor_mul(
                                    ctxS[m][bass.ts(half, 64), qtok],
                                    crs[half][0:64, :], psD[:])

                        # ---- output projection + RS for this chunk ----
                        for ti in range(4):
                            t = 4 * s + ti
                            oSb = p3o.tile([128, EMB], mybir.dt.bfloat16, name="oSb")
                            for e in range(4):
                                psO = ps_out.tile([128, 512], F32, name="psO")
                                for m in range(NPAIR):
                                    nc.tensor.matmul(
                                        psO[:],
                                        ctxS[m][:, bass.ts(t, 128)],
                                        wo_sb[:, m, bass.ts(e, 512)],
                                        start=(m == 0), stop=(m == NPAIR - 1))
                                nc.scalar.copy(oSb[:, bass.ts(e, 512)], psO[:])
                            nc.sync.dma_start(
                                out=oPart[bass.ts(t, 128), :], in_=oSb[:])
                        nc.gpsimd.collective_compute(
                            "ReduceScatter", mybir.AluOpType.add,
                            replica_groups=[[0, 1, 2, 3], [4, 5, 6, 7]],
                            ins=[oPart[bass.ts(s, S2), :]],
                            outs=[rsOut[s]])
                        nc.sync.dma_start(out=out_ext[s], in_=rsOut[s])

    nc.finalize()
    return nc


def _host_inputs(x, cos, sin, Wq, Wk, Wv, Wo, q_scale, k_scale):
    f = np.float32
    pm = np.zeros((64, 64), f)
    for k in range(32):
        pm[k, k + 32] = 1.0
    for k in range(32, 64):
        pm[k, k - 32] = -1.0
    cosT = np.ascontiguousarray(cos.T.astype(f))            # [64, S]
    sinT = sin.T.astype(f)
    sinP = np.ascontiguousarray(np.roll(sinT, -32, axis=0))  # s_pre[d]=sin[(d+32)%64]
    ind = np.zeros((NPAIR, 2, 64, 8), f)
    for m in range(NPAIR):
        ind[m, 0, :, 2 * m] = 1.0 / 64
        ind[m, 1, :, 2 * m + 1] = 1.0 / 64
    indk = np.zeros((2, 64, 2), f)
    indk[0, :, 0] = 1.0 / 64
    indk[1, :, 1] = 1.0 / 64
    maskadd = np.zeros((4, 128, S2), f)
    p = np.arange(128)[:, None]
    j = np.arange(S2)[None, :]
    for r in range(4):
        maskadd[r] = np.where(p + 128 * r <= j, 0.0, NEG)
    common = {
        "cosT": cosT, "sinP": sinP, "pm": pm,
        "qsr": np.ascontiguousarray(q_scale.astype(f)[None, :]),
        "ksr": np.ascontiguousarray(k_scale.astype(f)[None, :]),
        "ind": ind, "indk": indk,
        "ones64": np.ones((1, 64), f),
        "vones": np.ones((128, KB), f),
        "maskadd": maskadd,
        "ident": np.eye(128, dtype=f),
    }
    in_maps = []
    for c in range(8):
        b, tp = divmod(c, TP)
        heads = [8 * tp + m for m in range(NPAIR)] + \
                [8 * tp + 4 + m for m in range(NPAIR)]
        order = []
        for m in range(NPAIR):
            order += [heads[m], heads[NPAIR + m]]
        qcols = np.concatenate([np.arange(h * D, (h + 1) * D) for h in order])
        g0, g1 = 2 * tp, 2 * tp + 1
        kvcols = np.concatenate([
            np.arange(g0 * D, (g0 + 1) * D), np.arange(g1 * D, (g1 + 1) * D)])
        im = dict(common)
        im["xT"] = np.ascontiguousarray(x[b].T.astype(f))
        im["wq"] = np.ascontiguousarray(Wq[:, qcols].astype(f))
        im["wkv"] = np.ascontiguousarray(np.concatenate(
            [Wk[:, kvcols], Wv[:, kvcols]], axis=1).astype(f))
        worows = np.concatenate([np.arange(h * D, (h + 1) * D) for h in order])
        im["wo"] = np.ascontiguousarray(Wo[worows, :].astype(f))
        in_maps.append(im)
    return in_maps


def kernel(x, mask, cos, sin, Wq, Wk, Wv, Wo, q_scale, k_scale):
    global LAST_RESULT
    x = np.asarray(x)
    if "nc" not in _CACHE:
        _CACHE["nc"] = _build()
    nc = _CACHE["nc"]
    in_maps = _host_inputs(np.asarray(x), np.asarray(cos), np.asarray(sin),
                           np.asarray(Wq), np.asarray(Wk), np.asarray(Wv),
                           np.asarray(Wo), np.asarray(q_scale),
                           np.asarray(k_scale))
    res = run_bass_kernel_spmd(nc, in_maps, core_ids=list(range(8)))
    LAST_RESULT = res
    out = np.empty((B, S, EMB), np.float32)
    for c in range(8):
        b, tp = divmod(c, TP)
        mo = np.asarray(res.results[c]["myout"]).astype(np.float32)
        for ch in range(NS2):
            r0 = S2 * ch + 128 * tp
            out[b, r0:r0 + 128, :] = mo[ch]
    return out
